# revision 27
# baseline (speedup 1.0000x reference)
"""Block-diagonal compress kernel: out = blockdiag(A) @ W @ blockdiag(B).

Shapes (full): W [8192, 8192] f32, A_blocks [128, 64, 64], B_blocks [128, 64, 64].
Sharding: row-shard W / A over 8 cores (1024 rows = 16 A-blocks each);
B replicated.  Each core computes outT = (A_bd @ W_shard @ B_bd)^T and the
host transposes each shard back on gather.

W ships as single-plane bf16 (halves the dominant HBM read to 16 MB/core);
A ships as a bf16 hi/lo pair so A@Wh = Ah@Wh + Al@Wh is exact in A — only W
is rounded, ~1.7e-3 rel err against the 2e-2 gate.  The hi/lo pair also
keeps two matmuls per stationary W chunk, amortizing LDWEIGHTS (56ns/mm
vs 128ns for singletons).

Per-core dataflow (all sizes per core):
  step 1:  T = (A_bd @ W)^T computed 128-column-chunk-wise with W as the
           matmul *stationary* operand:  matmul(lhsT=W[128 rows, 128 cols],
           rhs=blockdiag(A_even^T, A_odd^T)) -> psum [128 cols, 128 rows].
           This absorbs the transpose that a chained matmul otherwise needs.
  step 2:  outT[chunk] = matmul(lhsT=blockdiag(B_j0, B_j1), rhs=T chunk)
           at float32r (full-rate fp32 for moving free dim >= 256).

DMA layout: W is host-retiled to [G, R, 128, 1024] so each W load is one
fully contiguous 256 KB HBM read.  W loads ride the SP HWDGE queue; outT
stores ride the queue of whichever engine (DVE or ACT) produced the SBUF
copy, so a store trigger never blocks on a cross-engine copy; the bpack
preload rides the otherwise-idle gpsimd SWDGE queue to keep warmup clear.
PSUM->SBUF copies are split ~50/50 between DVE and ACT.
"""

import bass_rust
import numpy as np

import concourse.bass as bass
import concourse.mybir as mybir
from concourse.bass_utils import run_bass_kernel_spmd
from concourse.tile import TileContext

F32 = mybir.dt.float32
F32R = mybir.dt.float32r
BF16 = mybir.dt.bfloat16

N_CORES = 8
D = 8192
BLK = 64
ROWS_PC = D // N_CORES  # 1024 rows of W / out per core

_HOIST_OPCODES = {"Matmult", "DMACopy", "TensorCopy", "Memset", "Activation", "Drain"}


def _hoist_excess_matmul_waits(nc: bass.Bass, max_waits: int = 1) -> None:
    """walrus's codegen for several instruction structs (fused-LDWEIGHTS
    matmul, DMA_DIRECT2D, ...) has few sync-wait slots ("Too many sync wait
    commands"). Move excess semaphore waits off such instructions into
    standalone EventSemaphore instructions right before them on the same
    engine queue — the sequencer executes those in order, so the instruction
    still starts only after all waits pass."""
    ctr = 0
    for fnc in nc.m.functions:
        for bb in fnc.blocks:
            new = []
            for ins in bb.instructions:
                si = ins.sync_info if ins.opcode in _HOIST_OPCODES else None
                if si is not None and len(si.on_wait) > max_waits:
                    waits = list(si.on_wait)
                    for w in waits[:-max_waits]:
                        evs = mybir.InstEventSemaphore(
                            name=f"mmwaithoist-{ctr}", ins=[], outs=[]
                        )
                        ctr += 1
                        evs.engine = ins.engine
                        evs.sync_info = bass_rust.SyncInfo(on_wait=[w], on_update=[])
                        new.append(evs)
                    ins.sync_info.on_wait = waits[-max_waits:]
                new.append(ins)
            bb.instructions[:] = new


def build_nc(rows_pc: int = ROWS_PC, d: int = D, hoist: bool = True) -> bass.Bass:
    """One-core SPMD program. rows_pc/d scaled down only for sim tests.
    hoist=False keeps waits on the original instructions (CoreSim's race
    detector wants every instruction to carry its own updates; the hoisted
    variant is for walrus, whose ISA structs have too few wait slots)."""
    R = rows_pc // 128  # 128-row slabs per core (= A-block pairs)
    n2 = (rows_pc + 511) // 512

    nc = bass.Bass()
    widths = [1024] * 7 + [768, 256]
    assert sum(widths) == d
    NG = len(widths)
    c0s = [sum(widths[:i]) for i in range(NG)]
    wq_ext = nc.declare_dram_parameter("wq", [R, 128, d], BF16, isOutput=False)
    ah_ext = nc.declare_dram_parameter("ah", [128, R * 128], BF16, isOutput=False)
    al_ext = nc.declare_dram_parameter("al", [128, R * 128], BF16, isOutput=False)
    bp_ext = nc.declare_dram_parameter("bpack", [128, d], F32R, isOutput=False)
    ot_ext = nc.declare_dram_parameter("outt", [d, rows_pc], BF16, isOutput=True)

    with TileContext(nc) as tc:
        with (
            tc.tile_pool(name="const", bufs=1) as cpool,
            tc.tile_pool(name="wp", bufs=8) as wpool,
            tc.tile_pool(name="tg", bufs=3) as tpool,
            tc.tile_pool(name="op", bufs=6) as opool,
            tc.tile_pool(name="p1", bufs=2, space="PSUM") as p1pool,
            tc.tile_pool(name="p2", bufs=2, space="PSUM") as p2pool,
        ):
            # A hi/lo on the scalar HWDGE queue (small; they gate the first
            # matmul); bpack rides the otherwise-idle gpsimd SWDGE queue so
            # neither the W-load nor the store queue pays for it.
            ah = cpool.tile([128, R * 128], BF16)
            nc.scalar.dma_start(out=ah[:], in_=ah_ext[:])
            al = cpool.tile([128, R * 128], BF16)
            nc.scalar.dma_start(out=al[:], in_=al_ext[:])
            # bpack arrives as per-group 512 KB chunks, each issued one group
            # ahead of its use: a single 4 MB preload monopolizes the DMA
            # engines for ~20us at t0 and starves the W-load stream.
            bptiles = [
                cpool.tile([128, widths[g]], F32R, name=f"bp{g}")
                for g in range(NG)
            ]
            nc.gpsimd.dma_start(
                out=bptiles[0][:], in_=bp_ext[:, 0 : widths[0]]
            )

            def step2_chunk(gp: int, cc: int, tgp):
                """(docstring below)"""
                """B-multiply + copy + store for 128-col chunk cc of group gp.
                Copy engine and store queue alternate by cc; the store trigger
                rides the queue attached to the copy's engine (ACT -> scalar
                HWDGE, DVE -> gpsimd SWDGE) so it never waits cross-engine."""
                j2 = c0s[gp] // 128 + cc
                lb = bptiles[gp][:, cc * 128 : (cc + 1) * 128]
                p2 = p2pool.tile([128, rows_pc], F32)
                for s in range(n2):
                    w0 = s * 512
                    w1 = min(rows_pc, w0 + 512)
                    ts = slice(cc * rows_pc + w0, cc * rows_pc + w1)
                    nc.tensor.matmul(
                        p2[:, w0:w1], lhsT=lb, rhs=tgp[:, ts],
                        start=True, stop=True,
                    )
                # store outT as bf16 (halves the 32 MB store stream);
                # host upcasts to f32. Rounding adds ~1.1e-3 rel err.
                ot = opool.tile([128, rows_pc], BF16)
                if cc % 2 == 0:
                    nc.scalar.copy(ot[:], p2[:])
                    nc.scalar.dma_start(
                        out=ot_ext[j2 * 128 : (j2 + 1) * 128, :], in_=ot[:]
                    )
                else:
                    nc.vector.tensor_copy(ot[:], p2[:])
                    nc.gpsimd.dma_start(
                        out=ot_ext[j2 * 128 : (j2 + 1) * 128, :], in_=ot[:]
                    )

            # Step 2 of group g-1 is software-pipelined INTO step 1 of group
            # g (chunk cc=r folded into slab r): stores and PSUM->SBUF copies
            # drain evenly through the whole group instead of bursting at
            # group boundaries, which kept colliding on the copy engines and
            # stalling PSUM recycling.
            prev_tg = None
            for g in range(NG):
                wd = widths[g]
                ncc = wd // 128
                pcc = widths[g - 1] // 128 if g > 0 else 0
                # T for this column group: col = c0s[g] + cc*128 + p at
                # free offset cc*rows_pc + r*128 + n for the core's rows.
                tg = tpool.tile([128, 8 * rows_pc], F32R)
                tgv = tg[:].rearrange("p (cc r n) -> p cc r n", cc=8, r=R)
                for r in range(R):
                    wt = wpool.tile([128, wd], BF16)
                    nc.sync.dma_start(
                        out=wt[:], in_=wq_ext[r, :, c0s[g] : c0s[g] + wd]
                    )
                    p1 = p1pool.tile([128, wd], F32)
                    for cc in range(ncc):
                        cs = slice(cc * 128, (cc + 1) * 128)
                        rs = slice(r * 128, (r + 1) * 128)
                        nc.tensor.matmul(
                            p1[:, cs], lhsT=wt[:, cs], rhs=ah[:, rs],
                            start=True, stop=False,
                        )
                        nc.tensor.matmul(
                            p1[:, cs], lhsT=wt[:, cs], rhs=al[:, rs],
                            start=False, stop=True,
                        )
                    src = p1[:].rearrange("p (cc n) -> p cc n", cc=ncc)
                    # tgv copy on the engine opposite to this slab's step-2
                    # chunk copy, so the per-slab copy load is one tile each.
                    if r % 2 == 0:
                        nc.vector.tensor_copy(tgv[:, 0:ncc, r, :], src)
                    else:
                        nc.scalar.copy(tgv[:, 0:ncc, r, :], src)
                    if prev_tg is not None and r < pcc:
                        step2_chunk(g - 1, r, prev_tg)
                    if r == 0 and g + 2 < NG:
                        # paced bpack chunk load: sits on the scalar queue
                        # behind the work above, so it dispatches early in
                        # group g instead of piling up at t0.
                        nc.scalar.dma_start(
                            out=bptiles[g + 2][:],
                            in_=bp_ext[
                                :, c0s[g + 2] : c0s[g + 2] + widths[g + 2]
                            ],
                        )
                    if r == 4 and g == 0:
                        # bp1 deferred out of the t0 window: the W-load
                        # stream owns warmup bandwidth.
                        nc.scalar.dma_start(
                            out=bptiles[1][:],
                            in_=bp_ext[:, c0s[1] : c0s[1] + widths[1]],
                        )
                prev_tg = tg
            for cc in range(widths[NG - 1] // 128):
                step2_chunk(NG - 1, cc, prev_tg)
    if hoist:
        _hoist_excess_matmul_waits(nc)
    return nc


def pack_at(a_blocks: np.ndarray) -> np.ndarray:
    """[2R, 64, 64] A blocks -> [128, R*128] with
    out[64*b + k, 128*r + 64*b + n] = A[2r+b][n, k] (transposed, pair-blockdiag)."""
    nb = a_blocks.shape[0]
    R = nb // 2
    out = np.zeros((128, R * 128), np.float32)
    at = a_blocks.transpose(0, 2, 1)
    out[0:64].reshape(64, R, 2, 64)[:, :, 0, :] = at[0::2].transpose(1, 0, 2)
    out[64:128].reshape(64, R, 2, 64)[:, :, 1, :] = at[1::2].transpose(1, 0, 2)
    return out


def pack_b(b_blocks: np.ndarray) -> np.ndarray:
    """[2J, 64, 64] B blocks -> [128, J*128] with
    out[64*b + k, 128*j + 64*b + n] = B[2j+b][k, n] (pair-blockdiag, untransposed)."""
    nb = b_blocks.shape[0]
    J = nb // 2
    out = np.zeros((128, J * 128), np.float32)
    out[0:64].reshape(64, J, 2, 64)[:, :, 0, :] = b_blocks[0::2].transpose(1, 0, 2)
    out[64:128].reshape(64, J, 2, 64)[:, :, 1, :] = b_blocks[1::2].transpose(1, 0, 2)
    return out


def pack_w(w_shard: np.ndarray):
    """[rows_pc, d] -> bf16 [R, 128, d]: slab-major, columns natural, so a
    (group, slab) W tile is a [128, width] slice with 16 KB line stride."""
    import ml_dtypes

    rows_pc, d = w_shard.shape
    R = rows_pc // 128
    return np.ascontiguousarray(
        w_shard.reshape(R, 128, d).astype(ml_dtypes.bfloat16)
    )


def split_bf16(x: np.ndarray):
    import ml_dtypes

    hi = x.astype(ml_dtypes.bfloat16)
    lo = (x - hi.astype(np.float32)).astype(ml_dtypes.bfloat16)
    return hi, lo


_NC_CACHE: dict = {}


def run(W, A_blocks, B_blocks, trace: bool = False, trace_cores=None):
    W = np.asarray(W, dtype=np.float32)
    A_blocks = np.asarray(A_blocks, dtype=np.float32)
    B_blocks = np.asarray(B_blocks, dtype=np.float32)
    assert W.shape == (D, D) and A_blocks.shape == (D // BLK, BLK, BLK)

    if "nc" not in _NC_CACHE:
        _NC_CACHE["nc"] = build_nc()
    nc = _NC_CACHE["nc"]

    bp = pack_b(B_blocks)
    in_maps = []
    for c in range(N_CORES):
        wq = pack_w(W[ROWS_PC * c : ROWS_PC * (c + 1)])
        ah, al = split_bf16(pack_at(A_blocks[16 * c : 16 * (c + 1)]))
        in_maps.append({"wq": wq, "ah": ah, "al": al, "bpack": bp})
    res = run_bass_kernel_spmd(nc, in_maps, core_ids=list(range(N_CORES)), trace=trace, trace_cores=trace_cores)
    out = np.empty((D, D), np.float32)
    for c in range(N_CORES):
        out[ROWS_PC * c : ROWS_PC * (c + 1), :] = res.results[c]["outt"].T.astype(
            np.float32
        )
    return out, res


def kernel(W, A_blocks, B_blocks):
    out, _ = run(W, A_blocks, B_blocks, trace=False)
    return out
